# revision 26
# baseline (speedup 1.0000x reference)
"""MinkowskiGlobalPooling (average=True) segment-mean kernel for 8 trn2 cores.

Full inputs in, full output out. batch_idx is sorted, so the segment sum is
32 contiguous-range sums. Strategy:
  - host quantizes feats to fp8 E3M4 (4 mantissa bits; pooled rel-err ~1.4e-2,
    under the 2e-2 gate),
  - rows of each batch are split evenly across the 8 cores; per core each
    batch's rows are padded to a whole number of 128-row groups (pad rows are
    all-zero so they add nothing to the sums),
  - every 128-row group therefore belongs to exactly ONE batch, so the
    matmul "mask" lhsT is one of 32 one-hot column tiles built on-device
    (no per-row mask generation, no index sideband),
  - runs of up to 8 same-batch groups pack into ONE wide matmul
    (rhs [128, s*64], psum [32, s*64] — 2KB, one PSUM bank) so LDWEIGHTS
    and instruction dispatch amortize; the 8 slot partial-sums fold on
    device before the tiny output DMA,
  - stream chunks alternate over the two HW-DGE DMA queues (scalar/sync)
    with ~55KB per-partition descriptor runs; this hides per-DMA
    descriptor-gen gaps and avoids SW-DGE interference — the kernel is
    HBM-bandwidth-bound (~330-400 GB/s/core),
  - host sums the 8 per-core partial strips and divides by counts (known
    host-side from the sharding split).
The group->batch schedule depends on the input's batch counts, so the Bass
program is built (and cached) per counts-signature inside kernel().
"""

import numpy as np


def _ensure_import_path():
    try:
        import concourse.bass  # noqa: F401
    except ImportError:
        import sys

        for p in ("/opt/trn_rl_repo", "/root/.axon_site/_ro/trn_rl_repo"):
            if p not in sys.path:
                sys.path.insert(0, p)


N_CORES = 8
B = 32  # batches
C = 64  # channels
P = 128  # SBUF partitions / matmul contraction
COL_GROUPS = 4
SLOTS = 8  # groups packed per matmul (8*C*4B = 2048B/partition, one PSUM bank)


def _make_schedule(n_groups):
    """Chunk the group sequence for 2 HW-DGE DMA queues. Matmuls gate on
    whole-chunk DMA completion, so chunk sizes DECREASE toward the end:
    big chunks early keep descriptor runs large (~44-59KB/partition, under
    the 64KB SDMA limit) while the geometric tail lets the PE's last packs
    start as soon as the (small) final transfers land instead of waiting on
    a 7.5MB chunk."""
    lead = [64, 96]
    r = n_groups - sum(lead)
    assert r > 0
    big = min(768, r // 5)  # 768*64B*4bufs = 192KB/partition SBUF
    nb = 4
    taper_total = r - nb * big
    taper = [taper_total - taper_total // 2, taper_total // 2]
    body = [big] * nb
    assert all(t > 0 for t in body + taper)
    return lead + body + taper


def build_program(schedule, group_batch):
    """Build the per-core Bass program. All cores run the identical program;
    only the stream contents differ per core."""
    _ensure_import_path()
    import concourse.mybir as mybir
    from concourse import bacc
    from concourse.tile import TileContext

    f32 = mybir.dt.float32
    f8 = mybir.dt.float8e3
    n = len(group_batch)
    assert sum(schedule) == n

    nc = bacc.Bacc()
    stream = nc.dram_tensor("stream", [P * n * C], f8, kind="ExternalInput")
    out = nc.dram_tensor("out", [COL_GROUPS * B, C], f32, kind="ExternalOutput")

    # pre-plan the matmul packs: runs of consecutive same-batch groups within
    # a chunk share one lhsT and stream as a single wide matmul
    packs = []  # (chunk_idx, j_in_chunk, s, batch)
    k = 0
    for ci, t in enumerate(schedule):
        j = 0
        while j < t:
            b = group_batch[k]
            s = 1
            while s < SLOTS and j + s < t and group_batch[k + s] == b:
                s += 1
            packs.append((ci, j, s, int(b)))
            j += s
            k += s
    n_packs = len(packs)

    with TileContext(nc) as tc:
        with (
            tc.tile_pool(name="const", bufs=1) as cpool,
            tc.tile_pool(name="feats", bufs=4) as fpool,
            tc.tile_pool(name="psum", bufs=1, space="PSUM") as ppool,
            tc.tile_pool(name="outp", bufs=1) as opool,
        ):
            # one-hot mask table built on-device: masks_sb[p, b*B + j] = (j==b),
            # i.e. ones at flat positions 33*b for b in 0..31
            masks_sb = cpool.tile([P, B * B], f8)
            nc.vector.memset(masks_sb[:], 0.0)
            nc.vector.memset(
                masks_sb[:, : 33 * 31].rearrange("p (b x) -> p b x", x=33)[
                    :, :, 0:1
                ],
                1.0,
            )
            nc.vector.memset(masks_sb[:, 33 * 31 : 33 * 31 + 1], 1.0)

            psum = ppool.tile([COL_GROUPS * B, SLOTS * C], f32)
            # Zero-valued "start" matmuls, one per column-group strip. All
            # real matmuls then accumulate (start=False), making the result
            # independent of the has_written-clear granularity.
            zero_mk = cpool.tile([P, B], f8)
            nc.vector.memset(zero_mk[:], 0.0)
            zslot = cpool.tile([P, SLOTS * C], f8)
            nc.vector.memset(zslot[:], 0.0)
            for g in range(COL_GROUPS):
                nc.tensor.matmul(
                    psum[g * B : (g + 1) * B, :],
                    lhsT=zero_mk[:],
                    rhs=zslot[:],
                    start=True,
                    stop=False,
                    tile_position=(0, g * B),
                    skip_group_check=True,
                )
            # stream chunks split over the two HW-DGE queues so each queue's
            # descriptor-gen/setup overlaps the other's transfers; assign
            # greedily by accumulated bytes to keep the queues byte-balanced
            queues = [nc.scalar, nc.sync]
            qload = [0, 0]
            qassign = []
            for t in schedule:
                qi = 0 if qload[0] <= qload[1] else 1
                qassign.append(qi)
                qload[qi] += t
            offs = np.concatenate([[0], np.cumsum(schedule)])
            pack_i = 0
            for ci, t in enumerate(schedule):
                ft = fpool.tile([P, t * C], f8, tag="ft")
                queues[qassign[ci]].dma_start(
                    out=ft[:],
                    in_=stream[P * offs[ci] * C : P * offs[ci + 1] * C].rearrange(
                        "(p x) -> p x", p=P
                    ),
                )
                while pack_i < n_packs and packs[pack_i][0] == ci:
                    _, j, s, b = packs[pack_i]
                    g = pack_i % COL_GROUPS
                    nc.tensor.matmul(
                        psum[g * B : (g + 1) * B, : s * C],
                        lhsT=masks_sb[:, b * B : (b + 1) * B],
                        rhs=ft[:, j * C : (j + s) * C],
                        start=False,
                        stop=(pack_i >= n_packs - COL_GROUPS),
                        tile_position=(0, g * B),
                        skip_group_check=True,
                    )
                    pack_i += 1
            # fold the SLOTS axis on-device: [128, 8, 64] -> [128, 64]
            out_sb = opool.tile([COL_GROUPS * B, C], f32)
            nc.vector.tensor_reduce(
                out=out_sb[:],
                in_=psum[:].rearrange("p (s c) -> p c s", s=SLOTS),
                axis=mybir.AxisListType.X,
                op=mybir.AluOpType.add,
            )
            nc.sync.dma_start(out=out[:, :], in_=out_sb[:])
    nc.finalize()
    return nc


def host_prep(feats, batch_idx):
    """Shard each (sorted) batch's rows across cores, pad each core-batch
    segment to whole 128-row groups, quantize to fp8 E3M4, and pack the
    per-core chunk-major streams.

    Returns (in_maps, schedule, group_batch, counts)."""
    import ml_dtypes

    f8 = ml_dtypes.float8_e3m4
    feats = np.ascontiguousarray(np.asarray(feats, dtype=np.float32))
    bi = np.asarray(batch_idx)
    n_rows, c = feats.shape
    assert c == C, c

    counts = np.bincount(bi, minlength=B).astype(np.int64)
    assert counts.sum() == n_rows
    starts = np.concatenate([[0], np.cumsum(counts)[:-1]])
    # per-batch per-core split points (proportional, exact cover)
    splits = [
        (starts[b] + counts[b] * np.arange(N_CORES + 1) // N_CORES)
        for b in range(B)
    ]
    core_cnt = np.array(
        [[splits[b][m + 1] - splits[b][m] for b in range(B)] for m in range(N_CORES)]
    )
    gb = (core_cnt.max(axis=0) + P - 1) // P  # groups per batch (shared)
    n_groups = int(gb.sum())
    group_batch = np.repeat(np.arange(B), gb).astype(np.int64)
    schedule = _make_schedule(n_groups)
    goffs = np.concatenate([[0], np.cumsum(gb)])

    q = feats.astype(f8)

    in_maps = []
    for m in range(N_CORES):
        A = np.zeros((P, n_groups, C), dtype=f8)
        for b in range(B):
            s, e = splits[b][m], splits[b][m + 1]
            cnt = int(e - s)
            g = int(gb[b])
            if g == 0:
                continue
            blk = np.zeros((g * P, C), dtype=f8)
            blk[:cnt] = q[s:e]
            A[:, goffs[b] : goffs[b] + g, :] = blk.reshape(g, P, C).transpose(1, 0, 2)
        # chunk-major flat layout: chunk j = [P, t_j, C] contiguous block
        flat = np.empty(P * n_groups * C, dtype=f8)
        pos = 0
        off = 0
        for t in schedule:
            blk = A[:, off : off + t, :]
            flat[pos : pos + blk.size] = blk.reshape(-1)
            pos += blk.size
            off += t
        in_maps.append({"stream": flat})
    return in_maps, schedule, group_batch, counts


_CACHED = {}


def get_program(schedule, group_batch):
    key = (tuple(schedule), group_batch.tobytes())
    if key not in _CACHED:
        _CACHED[key] = build_program(schedule, group_batch)
    return _CACHED[key]


def run_on_cores(in_maps, nc, trace=False):
    _ensure_import_path()
    from concourse.bass_utils import run_bass_kernel_spmd

    return run_bass_kernel_spmd(nc, in_maps, list(range(N_CORES)), trace=trace)


def finalize(per_core_outs, counts):
    acc = np.zeros((B, C), dtype=np.float64)
    for o in per_core_outs:
        o = np.asarray(o, dtype=np.float64)
        acc += o.reshape(-1, B, C).sum(axis=0)
    pooled = acc / np.maximum(counts, 1.0)[:, None]
    return pooled.astype(np.float32)


def kernel(feats, batch_idx, num_batches):
    assert int(num_batches) == B
    in_maps, schedule, group_batch, counts = host_prep(feats, batch_idx)
    nc = get_program(schedule, group_batch)
    res = run_on_cores(in_maps, nc)
    return finalize([r["out"] for r in res.results], counts)
